# revision 1
# baseline (speedup 1.0000x reference)
"""TRN2 Bass kernel for the GAT message-passing layer (8-core SPMD).

Contract: kernel(**inputs) takes the FULL inputs of reference.setup_inputs()
(x [50000,256] f32, src/dst [1600000] int64, Ws [8,64,256] f32,
As [8,128,1] f32) and returns the FULL output [50000, 512] f32.

Math: in the reference, e = Al[src] + Ar[dst] is softmaxed over segments of
src; the Al[src] term is constant within a segment and cancels, leaving
  out[i] = elu( sum_{j in N(i)} u_j * t_j / sum_{j in N(i)} u_j ),
  t[n,h,:] = x[n] @ Ws[h].T,  u = exp(x @ (Ws[h].T @ a_r[h])).

Device algorithm (identical program on 8 NeuronCores, src-range sharded):
  Phase 1: every core builds the Y table  Y[n] = [u*t (512) | u (8)]  (bf16)
    for all nodes via PE matmuls (f32r), into two DRAM half-tables (so gather
    indices fit int16).
  Phase 2: edges sorted by (128-node src window, dst-half), padded to
    128-edge blocks; per block, dma_gather fetches the 1280B Y rows, DVE
    builds a one-hot S[edge, src-slot] (fp8) by comparing the per-edge local
    src against an iota row, and the PE accumulates S^T @ [t~|u] into PSUM
    per window (segment sum).  Then normalize, ELU, store.
"""

import math

import numpy as np
import ml_dtypes

import concourse.bacc as bacc
import concourse.mybir as mybir
from concourse.tile import TileContext, add_dep_helper
from concourse.bass_utils import run_bass_kernel_spmd
from contextlib import ExitStack

F32 = mybir.dt.float32
F32R = mybir.dt.float32r
BF16 = mybir.dt.bfloat16
FP8 = mybir.dt.float8e4
I16 = mybir.dt.int16

P = 128
IN_FEAT = 256
HEADS = 8
OUT = 64
TD = HEADS * OUT  # 512
N_CORES = 8

LAST_EXEC_TIME_NS = None


class _Config:
    def __init__(self, n_nodes, src, dst, n_cores=N_CORES):
        self.n_cores = n_cores
        self.dt_y = BF16
        self.yw = 640  # 512 + 8 + 120 pad -> 1280B rows (mult of 256B)
        self.dt_meta = BF16
        self.dt_s = FP8

        self.n_nodes = n_nodes
        self.w_per_core = math.ceil(n_nodes / (n_cores * P))
        self.npc = self.w_per_core * P
        self.n_pad = self.npc * n_cores
        self.x_tiles = self.n_pad // P
        h0_tiles = self.x_tiles // 2
        assert h0_tiles * P < 32768 and (self.x_tiles - h0_tiles) * P < 32768
        self.h0_tiles = h0_tiles
        self.h0_rows = h0_tiles * P
        self.h1_rows = (self.x_tiles - h0_tiles) * P

        W = self.w_per_core
        src = np.asarray(src, dtype=np.int64)
        dst = np.asarray(dst, dtype=np.int64)
        core = src // self.npc
        w = (src % self.npc) // P
        lsrc = src % P
        half = (dst >= self.h0_rows).astype(np.int64)
        lidx = dst - self.h0_rows * half

        counts = np.zeros((n_cores, W, 2), dtype=np.int64)
        np.add.at(counts, (core, w, half), 1)
        cap = counts.max(axis=0)
        self.cap_blocks = np.ceil(cap / P).astype(np.int64)
        self.tot_blocks = int(self.cap_blocks.sum())
        self.tot_idx = self.tot_blocks * P
        self.ch_max = max(8, int(self.cap_blocks.max()))

        key = (core * W + w) * 2 + half
        order = np.argsort(key, kind="stable")
        s_core, s_w, s_half = core[order], w[order], half[order]
        s_lsrc, s_lidx = lsrc[order], lidx[order]

        blk_off = np.zeros((W, 2), dtype=np.int64)
        acc = 0
        for wi in range(W):
            for hi in range(2):
                blk_off[wi, hi] = acc
                acc += self.cap_blocks[wi, hi]

        gkey = (s_core * W + s_w) * 2 + s_half
        change = np.r_[True, gkey[1:] != gkey[:-1]]
        grp_start = np.flatnonzero(change)
        grp_id = np.cumsum(change) - 1
        grp_rank = np.arange(len(order)) - grp_start[grp_id]
        slot = blk_off[s_w, s_half] * P + grp_rank

        calls = []
        for wi in range(W):
            for hi in range(2):
                c = int(self.cap_blocks[wi, hi])
                b0 = int(blk_off[wi, hi])
                off = 0
                while off < c:
                    nb = min(self.ch_max, c - off)
                    calls.append((wi, hi, b0 + off, nb))
                    off += nb
        self.calls = calls

        self.idx_packed = np.zeros((n_cores, 128, self.tot_idx // 16), np.int16)
        self.meta_packed = np.full((n_cores, P, self.tot_blocks), -1.0, np.float32)
        call_starts = np.array([b0 * P for (_, _, b0, nb) in calls], dtype=np.int64)
        ci = np.searchsorted(call_starts, slot, side="right") - 1
        g0 = call_starts[ci]
        i_in_call = slot - g0
        self.idx_packed[s_core, i_in_call % 16, g0 // 16 + i_in_call // 16] = \
            s_lidx.astype(np.int16)
        # each of the 8 GpSimd cores reads indices from its own 16 partitions
        self.idx_packed[:, 16:, :] = np.tile(self.idx_packed[:, :16, :], (1, 7, 1))
        self.meta_packed[s_core, slot % P, slot // P] = s_lsrc.astype(np.float32)


def _build_program(cfg):
    nc = bacc.Bacc("TRN2", target_bir_lowering=False, debug=False,
                   num_devices=cfg.n_cores, num_swdge_queues=4)
    YW = cfg.yw
    DTY = cfg.dt_y
    W = cfg.w_per_core

    xt_d = nc.dram_tensor("xt", [IN_FEAT, cfg.n_pad], F32R, kind="ExternalInput")
    wcat_d = nc.dram_tensor("wcat", [IN_FEAT, TD], F32R, kind="ExternalInput")
    war_d = nc.dram_tensor("war", [IN_FEAT, HEADS], F32R, kind="ExternalInput")
    iota_d = nc.dram_tensor("iota", [P, P], cfg.dt_meta, kind="ExternalInput")
    idx_d = nc.dram_tensor("idx", [128, cfg.tot_idx // 16], I16, kind="ExternalInput")
    meta_d = nc.dram_tensor("meta", [P, cfg.tot_blocks], cfg.dt_meta,
                            kind="ExternalInput")
    out_d = nc.dram_tensor("out", [cfg.npc, TD], F32, kind="ExternalOutput")
    y0_d = nc.dram_tensor("y0", [cfg.h0_rows, YW], DTY, kind="Internal")
    y1_d = nc.dram_tensor("y1", [cfg.h1_rows, YW], DTY, kind="Internal")

    y_writes = [[], []]
    with TileContext(nc) as tc:
        # ---------------- phase 1: build Y table ----------------
        with ExitStack() as ctx:
            consts = ctx.enter_context(tc.tile_pool(name="consts", bufs=1))
            wc = consts.tile([P, 2, TD], F32R, tag="wc")
            nc.sync.dma_start(wc[:, :, :], wcat_d.rearrange("(c p) n -> p c n", p=P))
            wr = consts.tile([P, 2, HEADS], F32R, tag="wr")
            nc.sync.dma_start(wr[:, :, :], war_d.rearrange("(c p) n -> p c n", p=P))

            xin = ctx.enter_context(tc.tile_pool(name="xin", bufs=3))
            yout = ctx.enter_context(tc.tile_pool(name="yout", bufs=3))
            ps_t = ctx.enter_context(tc.tile_pool(name="ps_t", bufs=2, space="PSUM"))

            for t in range(cfg.x_tiles):
                xT = xin.tile([P, 2, P], F32R)
                nc.sync.dma_start(xT[:, 0, :], xt_d[0:P, t * P:(t + 1) * P])
                nc.sync.dma_start(xT[:, 1, :], xt_d[P:2 * P, t * P:(t + 1) * P])
                pt = ps_t.tile([P, TD], F32, tag="pt")
                par = ps_t.tile([P, HEADS], F32, tag="par")
                nc.tensor.matmul(pt[:, :], xT[:, 0, :], wc[:, 0, :], start=True, stop=False)
                nc.tensor.matmul(pt[:, :], xT[:, 1, :], wc[:, 1, :], start=False, stop=True)
                nc.tensor.matmul(par[:, :], xT[:, 0, :], wr[:, 0, :], start=True, stop=False)
                nc.tensor.matmul(par[:, :], xT[:, 1, :], wr[:, 1, :], start=False, stop=True)
                ysb = yout.tile([P, YW], DTY)
                nc.scalar.activation(ysb[:, TD:TD + HEADS], par[:, :],
                                     mybir.ActivationFunctionType.Exp)
                nc.vector.tensor_tensor(
                    ysb[:, 0:TD].rearrange("p (h o) -> p h o", h=HEADS),
                    pt[:, :].rearrange("p (h o) -> p h o", h=HEADS),
                    ysb[:, TD:TD + HEADS].unsqueeze(2).broadcast_to([P, HEADS, OUT]),
                    mybir.AluOpType.mult,
                )
                UC = TD + HEADS
                if t < cfg.h0_tiles:
                    wi_ = nc.sync.dma_start(y0_d[t * P:(t + 1) * P, 0:UC], ysb[:, 0:UC])
                    y_writes[0].append(wi_)
                else:
                    tt = t - cfg.h0_tiles
                    wi_ = nc.sync.dma_start(y1_d[tt * P:(tt + 1) * P, 0:UC], ysb[:, 0:UC])
                    y_writes[1].append(wi_)

        # ---------------- phase 2: gather + segment sums ----------------
        with ExitStack() as ctx:
            consts2 = ctx.enter_context(tc.tile_pool(name="consts2", bufs=1))
            iota = consts2.tile([P, P], cfg.dt_meta)
            nc.sync.dma_start(iota[:, :], iota_d[:, :])
            idx_sb = consts2.tile([128, cfg.tot_idx // 16], I16, tag="idx")
            nc.sync.dma_start(idx_sb[:, :], idx_d[:, :])
            meta_sb = consts2.tile([P, cfg.tot_blocks], cfg.dt_meta, tag="meta")
            nc.sync.dma_start(meta_sb[:, :], meta_d[:, :])
            neg1 = consts2.tile([P, 1], F32, tag="neg1")
            nc.vector.memset(neg1[:, :], -1.0)

            gpool = ctx.enter_context(tc.tile_pool(name="gath", bufs=2))
            spool = ctx.enter_context(tc.tile_pool(name="onehot", bufs=2))
            opool = ctx.enter_context(tc.tile_pool(name="outp", bufs=2))
            ps_num = ctx.enter_context(tc.tile_pool(name="ps_num", bufs=2, space="PSUM"))
            ps_den = ctx.enter_context(tc.tile_pool(name="ps_den", bufs=2, space="PSUM"))

            fence_pending = [True, True]
            qn = [0]
            calls_by_w = [[] for _ in range(W)]
            for (wi, hi, b0, nb) in cfg.calls:
                calls_by_w[wi].append((hi, b0, nb))

            for wi in range(W):
                wcalls = calls_by_w[wi]
                nblk_w = sum(nb for (_, _, nb) in wcalls)
                pnum = ps_num.tile([P, TD], F32)
                pden = ps_den.tile([P, HEADS], F32)
                bi = 0
                for (hi, b0, nb) in wcalls:
                    g = gpool.tile([P, cfg.ch_max, YW], DTY)
                    src_t = y0_d if hi == 0 else y1_d
                    g_inst = nc.gpsimd.dma_gather(
                        out_ap=g[:, 0:nb, :],
                        in_ap=src_t[:, :],
                        idxs_ap=idx_sb[:, b0 * 8:(b0 + nb) * 8],
                        num_idxs=nb * P,
                        num_idxs_reg=nb * P,
                        elem_size=YW,
                        single_packet=(nb * P <= 1024),
                        queue_num=qn[0],
                    )
                    qn[0] = (qn[0] + 1) % 4
                    if fence_pending[hi]:
                        # The gather's indexed DRAM read of the Y tables is
                        # invisible to Tile's dependency tracking; gathers run
                        # in order on GpSimd, so gating the first gather per
                        # half on that half's writes fences the phase.
                        for wr_ in y_writes[hi]:
                            add_dep_helper(g_inst.ins, wr_.ins,
                                           reason="gather reads Y table")
                        fence_pending[hi] = False
                    s = spool.tile([P, cfg.ch_max, P], cfg.dt_s)
                    nc.vector.tensor_tensor(
                        s[:, 0:nb, :],
                        meta_sb[:, b0:b0 + nb].unsqueeze(2).broadcast_to([P, nb, P]),
                        iota[:, :].unsqueeze(1).broadcast_to([P, nb, P]),
                        mybir.AluOpType.is_equal,
                    )
                    for j in range(nb):
                        st = (bi == 0)
                        sp = (bi == nblk_w - 1)
                        nc.tensor.matmul(pnum[:, :], s[:, j, :], g[:, j, 0:TD],
                                         start=st, stop=sp, skip_group_check=True)
                        nc.tensor.matmul(pden[:, :], s[:, j, :],
                                         g[:, j, TD:TD + HEADS],
                                         start=st, stop=sp, skip_group_check=True)
                        bi += 1
                # ---- evict window ----
                den = opool.tile([P, HEADS], F32, tag="den")
                nc.vector.tensor_scalar_add(den[:, :], pden[:, :], 1e-30)
                rden = opool.tile([P, HEADS], F32, tag="rden")
                nc.vector.reciprocal(rden[:, :], den[:, :])
                hout = opool.tile([P, TD], F32, tag="hout")
                nc.vector.tensor_tensor(
                    hout[:, :].rearrange("p (h o) -> p h o", h=HEADS),
                    pnum[:, :].rearrange("p (h o) -> p h o", h=HEADS),
                    rden[:, :].unsqueeze(2).broadcast_to([P, HEADS, OUT]),
                    mybir.AluOpType.mult,
                )
                # elu(z) = max(z,0) + exp(min(z,0)) - 1; min/exp on ScalarE to
                # dodge the DVE<->GpSimd shared-SBUF-port lock (min(z,0) =
                # -relu(-z), folded via activation scale=-1).
                xm = opool.tile([P, TD], F32, tag="xm")
                nc.scalar.activation(xm[:, :], hout[:, :],
                                     mybir.ActivationFunctionType.Relu, scale=-1.0)
                ex = opool.tile([P, TD], F32, tag="ex")
                nc.scalar.activation(ex[:, :], xm[:, :],
                                     mybir.ActivationFunctionType.Exp, scale=-1.0)
                fin = opool.tile([P, TD], F32, tag="fin")
                nc.vector.scalar_tensor_tensor(
                    out=fin[:, :], in0=hout[:, :], scalar=0.0, in1=ex[:, :],
                    op0=mybir.AluOpType.max, op1=mybir.AluOpType.add,
                )
                fin2 = opool.tile([P, TD], F32, tag="fin2")
                nc.scalar.activation(fin2[:, :], fin[:, :],
                                     mybir.ActivationFunctionType.Identity,
                                     bias=neg1[:, :])
                nc.sync.dma_start(out_d[wi * P:(wi + 1) * P, :], fin2[:, :])

    nc.compile()
    return nc


def _round_f32r(a):
    b = np.ascontiguousarray(a, dtype=np.float32)
    b.view(np.uint32).__iand__(np.uint32(0xFFFFF000))
    return b


def kernel(x, src, dst, Ws, As):
    global LAST_EXEC_TIME_NS
    x = np.asarray(x, np.float32)
    src = np.asarray(src)
    dst = np.asarray(dst)
    Ws = np.asarray(Ws, np.float32)
    As = np.asarray(As, np.float32)
    n = x.shape[0]

    cfg = _Config(n, src, dst)
    nc = _build_program(cfg)

    xt = np.zeros((IN_FEAT, cfg.n_pad), np.float32)
    xt[:, :n] = x.T
    xt = _round_f32r(xt)
    wcat = _round_f32r(Ws.transpose(2, 0, 1).reshape(IN_FEAT, TD))
    a_r = As[:, OUT:, 0]
    war = _round_f32r(np.einsum("hof,ho->fh", Ws, a_r))
    iota = np.tile(np.arange(P, dtype=np.float32), (P, 1)).astype(ml_dtypes.bfloat16)
    meta = cfg.meta_packed.astype(ml_dtypes.bfloat16)

    in_maps = []
    for c in range(cfg.n_cores):
        in_maps.append({
            "xt": xt, "wcat": wcat, "war": war,
            "iota": np.ascontiguousarray(iota),
            "idx": np.ascontiguousarray(cfg.idx_packed[c]),
            "meta": np.ascontiguousarray(meta[c]),
        })

    res = run_bass_kernel_spmd(nc, in_maps, core_ids=list(range(cfg.n_cores)))
    LAST_EXEC_TIME_NS = res.exec_time_ns
    out = np.concatenate([res.results[c]["out"] for c in range(cfg.n_cores)],
                         axis=0)[:n]
    return np.ascontiguousarray(out, dtype=np.float32)
